# revision 1
# baseline (speedup 1.0000x reference)
"""CrossViewTransformer Trainium2 kernel.

Math (per batch b):
    q = Wq @ bev + bq          [D=8,  N=9216]
    k = Wk @ rv  + bk          [8,  N]
    v = Wv @ rv  + bv          [64, N]
    E[j, i] = k[:, j] . q[:, i]            (energy, rows=key pixel j, cols=query pixel i)
    A = softmax over i of E[j, :]
    z[:, j] = sum_i A[j, i] * v[:, i]
    out = bev + z

Sharding: 8 cores = 2 batches x 4 j-slabs of 2304 columns. Each core computes
softmax over the full i axis for its j slab; no collectives.

Device layout (per core):
    E^T tiles [i-chunk=128, j-block] from matmul(lhsT=q[:, i-tile], rhs=k[:, jblk])
    P^T = exp(E^T) via ScalarE (logits are O(5), no max subtraction needed), bf16
    Z[c(+ones row), jblk] += matmul(lhsT=v^T_ext[i-chunk, 65], rhs=P^T chunk)
    row 64 of Z = softmax denominators (ones column trick in v^T_ext)
    z = Z[0:64] * broadcast(1/Z[64]);  out = z + bev_residual

All matmuls in bf16 (1 cycle/col vs 4 for fp32), fp32 PSUM accumulation.
"""

import sys

if "/opt/trn_rl_repo" not in sys.path:
    sys.path.insert(0, "/opt/trn_rl_repo")

import os

import numpy as np
import ml_dtypes

VARIANT = os.environ.get("KERNEL_VARIANT", "full")  # full | noexp | noz
# Offload exp of every DVE_EXP-th group to DVE (0 = all exps on ScalarE)
DVE_EXP = int(os.environ.get("DVE_EXP", "3"))
MMW = int(os.environ.get("MMW", "512"))  # matmul moving width (512 or 1024 bf16)

B, C, H, W = 2, 64, 96, 96
N = H * W            # 9216
D = C // 8           # 8
NT = N // 128        # 72 i-chunks
NCORES = 8
JS = N // 4          # 2304 columns per core
# j-blocks per core; i-chunks are merged per exp call so every ACT call is
# [128, 1024] (g = 1024/jbw chunks per call). jbw=512 keeps Z at one PSUM bank,
# freeing banks for a 3-deep E-tile rotation (pipeline elasticity).
JBLOCKS = [(0, 512), (512, 512), (1024, 512), (1536, 512), (2048, 256)]

BF16 = ml_dtypes.bfloat16

_PROGRAMS = {}


def _enable_ldw_opt():
    """Recompile with walrus LDW dedup: consecutive matmuls sharing a
    stationary operand skip the redundant LDWEIGHTS (saves ~100ns/group)."""
    from concourse import bass_utils as bu

    if getattr(bu, "_ldw_patched", False):
        return
    orig = bu.run_command

    def patched(argv, **kwargs):
        argv = [
            "--enable-ldw-opt=true" if a == "--enable-ldw-opt=false" else a
            for a in argv
        ]
        return orig(argv, **kwargs)

    bu.run_command = patched
    bu._ldw_patched = True


def _build_program(reps=1, dve_exp=None):
    dve_exp = DVE_EXP if dve_exp is None else dve_exp
    import concourse.bacc as bacc
    import concourse.mybir as mybir
    from concourse import tile

    F32 = mybir.dt.float32
    BF = mybir.dt.bfloat16
    I16 = mybir.dt.int16
    Exp = mybir.ActivationFunctionType.Exp
    # bf16 Schraudolph fast-exp constants: bits16 = trunc(x * 128/ln2 + B);
    # int16 bit pattern reinterpreted as bf16 gives exp(x) to ~3% (end-to-end
    # effect ~1e-5 through softmax; validated vs reference). Used to offload a
    # quarter of the exp work from the bottleneck ScalarE to the idle DVE.
    EXP_A = float(128.0 / np.log(2.0))
    EXP_B = 16256.0 - 5.0

    if int(os.environ.get("LDW_OPT", "0")):
        # Off by default: walrus's ldw-opt pass crashes on this kernel.
        _enable_ldw_opt()

    nc = bacc.Bacc("TRN2", target_bir_lowering=False, num_devices=NCORES)

    rv_d = nc.dram_tensor("rv_ext", [65, N], BF, kind="ExternalInput")
    bev_d = nc.dram_tensor("bev_ext", [65, N], BF, kind="ExternalInput")
    rvs_d = nc.dram_tensor("rv_slab", [65, JS], BF, kind="ExternalInput")
    bres_d = nc.dram_tensor("bev_res", [C, JS], F32, kind="ExternalInput")
    wq_d = nc.dram_tensor("wq_ext", [65, D], BF, kind="ExternalInput")
    wk_d = nc.dram_tensor("wk_ext", [65, D], BF, kind="ExternalInput")
    wv_d = nc.dram_tensor("wv_ext", [65, 65], BF, kind="ExternalInput")
    out_d = nc.dram_tensor("out", [C, JS], F32, kind="ExternalOutput")

    with tile.TileContext(nc) as tc:
        with (
            tc.tile_pool(name="const", bufs=1) as cpool,
            tc.tile_pool(name="work", bufs=2) as wpool,
            tc.tile_pool(name="ptile", bufs=6) as ppool,
            tc.tile_pool(name="psum_e", bufs=3, space="PSUM") as epool,
            tc.tile_pool(name="psum_z", bufs=2, space="PSUM") as zpool,
        ):
          for _rep in range(reps):
            # ---- load inputs ----
            rv_sb = cpool.tile([65, N], BF, tag="rv")
            bev_sb = cpool.tile([65, N], BF, tag="bev")
            rvs_sb = cpool.tile([65, JS], BF, tag="rvs")
            bres_sb = cpool.tile([C, JS], F32, tag="bres")
            wq_sb = cpool.tile([65, D], BF, tag="wq")
            wk_sb = cpool.tile([65, D], BF, tag="wk")
            wv_sb = cpool.tile([65, 65], BF, tag="wv")

            nc.sync.dma_start(wq_sb[:], wq_d[:])
            nc.sync.dma_start(wk_sb[:], wk_d[:])
            nc.sync.dma_start(wv_sb[:], wv_d[:])
            nc.sync.dma_start(rvs_sb[:], rvs_d[:])
            for cix in range(4):
                s = slice(cix * JS, (cix + 1) * JS)
                nc.sync.dma_start(bev_sb[:, s], bev_d[:, s])
            for cix in range(4):
                s = slice(cix * JS, (cix + 1) * JS)
                nc.sync.dma_start(rv_sb[:, s], rv_d[:, s])
            nc.sync.dma_start(bres_sb[:], bres_d[:])

            # ---- projections ----
            # Copies PSUM->SBUF alternate between DVE and the (idle during
            # prologue) ScalarE to halve the startup critical path.
            q_sb = cpool.tile([D, N], BF, tag="q")     # lhsT tiles for energy
            k_sb = cpool.tile([D, JS], BF, tag="k")    # energy rhs (this core's slab)
            vt_sb = cpool.tile([128, NT * 65], BF, tag="vt")  # v^T_ext chunks

            def pcopy(i, out, in_):
                if i % 2 == 0:
                    nc.vector.tensor_copy(out, in_)
                else:
                    nc.scalar.copy(out, in_)

            for i, blk0 in enumerate(range(0, JS, 512)):
                pw = min(512, JS - blk0)
                ps = epool.tile([D, 512], F32, tag="e")
                nc.tensor.matmul(
                    ps[:, :pw], wk_sb[:], rvs_sb[:, blk0 : blk0 + pw],
                    start=True, stop=True,
                )
                pcopy(i, k_sb[:, blk0 : blk0 + pw], ps[:, :pw])

            for blk in range(N // 512):
                s = slice(blk * 512, (blk + 1) * 512)
                ps = epool.tile([D, 512], F32, tag="e")
                nc.tensor.matmul(ps[:], wq_sb[:], bev_sb[:, s], start=True, stop=True)
                pcopy(blk, q_sb[:, s], ps[:])

            for tg in range(NT // 4):   # 4 v^T chunks per PSUM tile / copy
                ps = epool.tile([128, 4 * 65], F32, tag="e")
                for m in range(4):
                    t = tg * 4 + m
                    nc.tensor.matmul(
                        ps[:, m * 65 : (m + 1) * 65],
                        rv_sb[:, t * 128 : (t + 1) * 128], wv_sb[:],
                        start=True, stop=True,
                    )
                pcopy(tg, vt_sb[:, tg * 4 * 65 : (tg + 1) * 4 * 65], ps[:])

            # ---- main attention loop ----
            if VARIANT == "prologue":
                nc.sync.dma_start(out_d[:], bres_sb[:])
                continue
            dummy_p = None
            if VARIANT == "noexp":
                dummy_p = cpool.tile([128, 1024], BF, tag="dummy_p")
                nc.vector.memset(dummy_p[:], 0.0)
            for jb0, jbw in JBLOCKS:
                g = 1024 // jbw          # i-chunks merged per exp call (1 or 4)
                ng = NT // g
                z_ps = zpool.tile([65, jbw], F32, tag="z")

                def z_mms(p_tile, grp):
                    for m in range(g):
                        t = grp * g + m
                        for pc0 in range(0, jbw, MMW):
                            pw = min(MMW, jbw - pc0)
                            rhs = p_tile[:, m * jbw + pc0 : m * jbw + pc0 + pw]
                            if rhs.dtype == I16:
                                rhs = rhs.bitcast(BF)
                            nc.tensor.matmul(
                                z_ps[:, pc0 : pc0 + pw],
                                vt_sb[:, t * 65 : (t + 1) * 65],
                                rhs,
                                start=(t == 0),
                                stop=(t == NT - 1),
                            )

                # Software-pipelined with a deep skew: PE runs energy(g),
                # ACT/DVE exp(g-1), PE z(g-3) — exp input is ready a group
                # early and P output has a full extra group of slack before
                # its z-matmul, hiding the DVE fast-exp latency (~2.2us).
                e_tiles = {}
                p_tiles = {}
                for grp in range(ng + 3):
                    if grp < ng:
                        e_ps = epool.tile([128, 1024], F32, tag="e")
                        e_tiles[grp] = e_ps
                        for m in range(g):
                            t = grp * g + m
                            for pc0 in range(0, jbw, MMW):
                                pw = min(MMW, jbw - pc0)
                                nc.tensor.matmul(
                                    e_ps[:, m * jbw + pc0 : m * jbw + pc0 + pw],
                                    q_sb[:, t * 128 : (t + 1) * 128],
                                    k_sb[:, jb0 + pc0 : jb0 + pc0 + pw],
                                    start=True, stop=True,
                                )
                    if 0 <= grp - 1 < ng:
                        e_prev = e_tiles.pop(grp - 1)
                        if VARIANT == "noexp":
                            p_sb = dummy_p
                        elif dve_exp and (grp - 1) % dve_exp == 1:
                            # DVE fast-exp: (E*A+B) -> trunc to int16 -> bf16 bits
                            t_sb = wpool.tile([128, 1024], F32, tag="tx")
                            nc.vector.tensor_scalar(
                                t_sb[:], e_prev[:], EXP_A, EXP_B,
                                mybir.AluOpType.mult, mybir.AluOpType.add,
                            )
                            p_sb = ppool.tile([128, 1024], I16, tag="p")
                            nc.vector.tensor_copy(p_sb[:], t_sb[:])
                        else:
                            p_sb = ppool.tile([128, 1024], BF, tag="p")
                            nc.scalar.activation(p_sb[:], e_prev[:], Exp)
                        p_tiles[grp - 1] = p_sb
                    if 0 <= grp - 3 < ng:
                        pt = p_tiles.pop(grp - 3)
                        if VARIANT != "noz":
                            z_mms(pt, grp - 3)

                # ---- normalize + residual + store ----
                if VARIANT == "noz":
                    nc.sync.dma_start(
                        out_d[:, jb0 : jb0 + jbw], bres_sb[:, jb0 : jb0 + jbw]
                    )
                    continue
                r_sb = wpool.tile([1, jbw], F32, tag="r")
                nc.vector.reciprocal(r_sb[:], z_ps[64:65, :])
                bc_sb = wpool.tile([C, jbw], F32, tag="bc")
                nc.gpsimd.partition_broadcast(bc_sb[:], r_sb[:])
                zn_sb = wpool.tile([C, jbw], F32, tag="zn")
                nc.vector.tensor_mul(zn_sb[:], z_ps[0:64, :], bc_sb[:])
                o_sb = wpool.tile([C, jbw], F32, tag="o")
                nc.vector.tensor_add(o_sb[:], zn_sb[:], bres_sb[:, jb0 : jb0 + jbw])
                nc.sync.dma_start(out_d[:, jb0 : jb0 + jbw], o_sb[:])

    nc.compile()
    return nc


def get_program(reps=1, dve_exp=None):
    key = (reps, dve_exp)
    if key not in _PROGRAMS:
        _PROGRAMS[key] = _build_program(reps, dve_exp)
    return _PROGRAMS[key]


def make_in_maps(rv_x, bev_x, Wq, bq, Wk, bk, Wv, bv):
    rv_x = np.asarray(rv_x, np.float32)
    bev_x = np.asarray(bev_x, np.float32)
    ones = np.ones((1, N), np.float32)
    wq_ext = np.concatenate([np.asarray(Wq).T, np.asarray(bq)[None]], 0).astype(BF16)
    wk_ext = np.concatenate([np.asarray(Wk).T, np.asarray(bk)[None]], 0).astype(BF16)
    wv_ext = np.zeros((65, 65), np.float32)
    wv_ext[:64, :64] = np.asarray(Wv).T
    wv_ext[64, :64] = np.asarray(bv)
    wv_ext[64, 64] = 1.0
    wv_ext = wv_ext.astype(BF16)

    in_maps = []
    for core in range(NCORES):
        b = core // 4
        j0 = (core % 4) * JS
        rv2 = rv_x[b].reshape(C, N)
        bev2 = bev_x[b].reshape(C, N)
        rv_ext = np.concatenate([rv2, ones], 0).astype(BF16)
        bev_ext = np.concatenate([bev2, ones], 0).astype(BF16)
        in_maps.append(
            {
                "rv_ext": rv_ext,
                "bev_ext": bev_ext,
                "rv_slab": np.ascontiguousarray(rv_ext[:, j0 : j0 + JS]),
                "bev_res": np.ascontiguousarray(bev2[:, j0 : j0 + JS]),
                "wq_ext": wq_ext,
                "wk_ext": wk_ext,
                "wv_ext": wv_ext,
            }
        )
    return in_maps


def run(inputs, trace=False, trace_kwargs=None, reps=1, in_maps=None):
    """Run on all 8 cores; returns (output ndarray, BassKernelResults)."""
    from concourse.bass_utils import run_bass_kernel_spmd

    nc = get_program(reps)
    if in_maps is None:
        in_maps = make_in_maps(**inputs)
    res = run_bass_kernel_spmd(
        nc,
        in_maps,
        core_ids=list(range(NCORES)),
        trace=trace,
        **(trace_kwargs or {}),
    )
    out = np.zeros((B, C, N), np.float32)
    for core in range(NCORES):
        b = core // 4
        j0 = (core % 4) * JS
        out[b, :, j0 : j0 + JS] = res.results[core]["out"]
    return out.reshape(B, C, H, W), res


def kernel(**inputs):
    out, _ = run(inputs)
    return out



# revision 18
# speedup vs baseline: 21.4683x; 21.4683x over previous
"""CrossViewTransformer Trainium2 kernel.

Math (per batch b):
    q = Wq @ bev + bq          [D=8,  N=9216]
    k = Wk @ rv  + bk          [8,  N]
    v = Wv @ rv  + bv          [64, N]
    E[j, i] = k[:, j] . q[:, i]            (energy, rows=key pixel j, cols=query pixel i)
    A = softmax over i of E[j, :]
    z[:, j] = sum_i A[j, i] * v[:, i]
    out = bev + z
Sharding: 8 cores = 2 batches x 4 j-slabs of 2304 columns. Each core computes
softmax over the full i axis for its j slab; no collectives.

Device layout (per core):
    E^T tiles [i-chunk=128, j-block] from matmul(lhsT=q[:, i-tile], rhs=k[:, jblk])
    P^T = exp(E^T): ScalarE real exp and DVE Schraudolph fast-exp alternate
    50/50 — the DVE path is a SINGLE tensor_scalar (f32 PSUM -> int16 bits of
    bf16 exp), which doubles DVE exp capacity vs a scalar+copy pair.
    Z[c(+ones row), jblk] += matmul(lhsT=v^T_ext[i-chunk, 65], rhs=P^T chunk)
    row 64 of Z = softmax denominators (ones column trick in v^T_ext)
    z = Z[0:64] * broadcast(1/Z[64]);  out = z + bev_residual

All matmuls bf16 (1 cycle/col), fp32 PSUM accumulation. Emission batches GB
groups of energy MMs / exps / z MMs so the PE stays in one tiling mode per
run (energy (32,128)-tile vs z (128,128)-tile mode switches drain the array).

Note: 4x row-tiled energy (tile_position row offsets 32/64/96, measured 3x
in isolation per the TRN2 docs) compiles and passes CoreSim but crashes HW
execution through this axon path for any nonzero row offset — reverted.
"""

import sys

if "/opt/trn_rl_repo" not in sys.path:
    sys.path.insert(0, "/opt/trn_rl_repo")

import os

import numpy as np
import ml_dtypes

VARIANT = os.environ.get("KERNEL_VARIANT", "full")  # full | noexp | noz
# Offload exp of every DVE_EXP-th group to DVE (0 = all exps on ScalarE)
DVE_EXP = int(os.environ.get("DVE_EXP", "2"))
MMW = int(os.environ.get("MMW", "512"))  # matmul moving width
GB = int(os.environ.get("GB", "2"))      # groups per emission batch
EXP1OP = int(os.environ.get("EXP1OP", "1"))  # DVE fast-exp as single op
NSTRIP = int(os.environ.get("NSTRIP", "4"))  # PE row strips for energy (2-4)
ER = int(os.environ.get("ER", "32"))  # energy AP rows (32: full strip, 8: D)
PADW = 32 if NSTRIP >= 3 else 64  # strip pitch / projection pad width
COVER = PADW * NSTRIP  # partitions covered by striped q/k layouts

B, C, H, W = 2, 64, 96, 96
N = H * W            # 9216
D = C // 8           # 8
NT = N // 128        # 72 i-chunks
NCORES = 8
JS = N // 4          # 2304 columns per core
JBLOCKS = [(0, 512), (512, 512), (1024, 512), (1536, 512), (2048, 256)]

BF16 = ml_dtypes.bfloat16

_PROGRAMS = {}


def _build_program(reps=1, dve_exp=None):
    dve_exp = DVE_EXP if dve_exp is None else dve_exp
    import concourse.bacc as bacc
    import concourse.mybir as mybir
    from concourse import tile

    F32 = mybir.dt.float32
    BF = mybir.dt.bfloat16
    I16 = mybir.dt.int16
    Exp = mybir.ActivationFunctionType.Exp
    # bf16 Schraudolph fast-exp constants: bits16 = trunc(x * 128/ln2 + B);
    # int16 bit pattern reinterpreted as bf16 gives exp(x) to ~3% (end-to-end
    # effect ~1e-5 through softmax; validated vs reference).
    EXP_A = float(128.0 / np.log(2.0))
    EXP_B = 16256.0 - 5.0

    nc = bacc.Bacc("TRN2", target_bir_lowering=False, num_devices=NCORES)

    rv_d = nc.dram_tensor("rv_ext", [65, N], BF, kind="ExternalInput")
    bev_d = nc.dram_tensor("bev_ext", [65, N], BF, kind="ExternalInput")
    rvs_d = nc.dram_tensor("rv_slab", [65, JS], BF, kind="ExternalInput")
    bres_d = nc.dram_tensor("bev_res", [C, JS], F32, kind="ExternalInput")
    # q/k weights padded to PADW out-channels (zeros) so each col-tiled
    # projection matmul fills its whole strip — keeps the PSUM tile fully
    # initialized for one-shot wide copies, and lets energy matmuls use
    # full-strip (zero-padded) contraction APs.
    wq_d = nc.dram_tensor("wq_ext", [65, PADW], BF, kind="ExternalInput")
    wk_d = nc.dram_tensor("wk_ext", [65, PADW], BF, kind="ExternalInput")
    wv_d = nc.dram_tensor("wv_ext", [65, 65], BF, kind="ExternalInput")
    out_d = nc.dram_tensor("out", [C, JS], F32, kind="ExternalOutput")

    # i-chunk t -> PE row strip PADW*(t % NSTRIP), q4 col block t // NSTRIP
    def strip(t):
        return PADW * (t % NSTRIP)

    QW = (NT // NSTRIP) * 128  # q4 column extent

    with tile.TileContext(nc) as tc:
        with (
            tc.tile_pool(name="const", bufs=1) as cpool,
            tc.tile_pool(name="work", bufs=2) as wpool,
            tc.tile_pool(name="ptile", bufs=6) as ppool,
            tc.tile_pool(name="psum_e", bufs=3, space="PSUM") as epool,
            tc.tile_pool(name="psum_z", bufs=2, space="PSUM") as zpool,
        ):
          for _rep in range(reps):
            # ---- load inputs ----
            rv_sb = cpool.tile([65, N], BF, tag="rv")
            bev_sb = cpool.tile([65, N], BF, tag="bev")
            rvs_sb = cpool.tile([65, JS], BF, tag="rvs")
            bres_sb = cpool.tile([C, JS], F32, tag="bres")
            wq_sb = cpool.tile([65, PADW], BF, tag="wq")
            wk_sb = cpool.tile([65, PADW], BF, tag="wk")
            wv_sb = cpool.tile([65, 65], BF, tag="wv")

            nc.sync.dma_start(wq_sb[:], wq_d[:])
            nc.sync.dma_start(wk_sb[:], wk_d[:])
            nc.sync.dma_start(wv_sb[:], wv_d[:])
            nc.sync.dma_start(rvs_sb[:], rvs_d[:])
            for cix in range(4):
                s = slice(cix * JS, (cix + 1) * JS)
                nc.sync.dma_start(bev_sb[:, s], bev_d[:, s])
            for cix in range(4):
                s = slice(cix * JS, (cix + 1) * JS)
                nc.sync.dma_start(rv_sb[:, s], rv_d[:, s])
            nc.sync.dma_start(bres_sb[:], bres_d[:])

            # ---- projections ----
            # (Row-tiled energy layouts were tried and crash HW execution —
            # tile_position with nonzero row offset; baseline (0,0) layout
            # retained.)
            q_sb = cpool.tile([D, N], BF, tag="q")     # lhsT tiles for energy
            k_sb = cpool.tile([D, JS], BF, tag="k")    # energy rhs (this slab)
            vt_sb = cpool.tile([128, NT * 65], BF, tag="vt")  # v^T_ext chunks

            def pcopy(i, out, in_):
                if i % 2 == 0:
                    nc.vector.tensor_copy(out, in_)
                else:
                    nc.scalar.copy(out, in_)

            ncopy = 0
            for blk in range(5):   # k projection
                b0 = blk * 512
                w = min(512, JS - b0)
                ps = epool.tile([PADW, 512], F32, tag="e")
                nc.tensor.matmul(
                    ps[:, :w], wk_sb[:], rvs_sb[:, b0 : b0 + w],
                    start=True, stop=True,
                )
                pcopy(ncopy, k_sb[:, b0 : b0 + w], ps[0:D, :w])
                ncopy += 1

            for blk in range(N // 512):   # q projection
                s = slice(blk * 512, (blk + 1) * 512)
                ps = epool.tile([PADW, 512], F32, tag="e")
                nc.tensor.matmul(ps[:], wq_sb[:], bev_sb[:, s],
                                 start=True, stop=True)
                pcopy(ncopy, q_sb[:, s], ps[0:D, :])
                ncopy += 1

            for tg in range(NT // 4):   # 4 v^T chunks per PSUM tile / copy
                ps = epool.tile([128, 4 * 65], F32, tag="e")
                for m in range(4):
                    t = tg * 4 + m
                    nc.tensor.matmul(
                        ps[:, m * 65 : (m + 1) * 65],
                        rv_sb[:, t * 128 : (t + 1) * 128], wv_sb[:],
                        start=True, stop=True,
                    )
                pcopy(ncopy, vt_sb[:, tg * 4 * 65 : (tg + 1) * 4 * 65], ps[:])
                ncopy += 1

            # ---- main attention loop ----
            if VARIANT == "prologue":
                nc.sync.dma_start(out_d[:], bres_sb[:])
                continue
            dummy_p = None
            if VARIANT == "noexp":
                dummy_p = cpool.tile([128, 1024], BF, tag="dummy_p")
                nc.vector.memset(dummy_p[:], 0.0)
            for jb0, jbw in JBLOCKS:
                g = 1024 // jbw          # i-chunks per exp group (2 or 4)
                ng = NT // g
                z_ps = zpool.tile([65, jbw], F32, tag="z")

                def e_mms(e_ps, grp):
                    for m in range(g):
                        t = grp * g + m
                        for pc0 in range(0, jbw, MMW):
                            pw = min(MMW, jbw - pc0)
                            nc.tensor.matmul(
                                e_ps[:, m * jbw + pc0 : m * jbw + pc0 + pw],
                                q_sb[:, t * 128 : (t + 1) * 128],
                                k_sb[:, jb0 + pc0 : jb0 + pc0 + pw],
                                start=True, stop=True,
                            )

                def z_mms(p_tile, grp):
                    for m in range(g):
                        t = grp * g + m
                        for pc0 in range(0, jbw, MMW):
                            pw = min(MMW, jbw - pc0)
                            rhs = p_tile[:, m * jbw + pc0 : m * jbw + pc0 + pw]
                            if rhs.dtype == I16:
                                rhs = rhs.bitcast(BF)
                            nc.tensor.matmul(
                                z_ps[:, pc0 : pc0 + pw],
                                vt_sb[:, t * 65 : (t + 1) * 65],
                                rhs,
                                start=(t == 0),
                                stop=(t == NT - 1),
                            )

                def do_exp(grp):
                    e_prev = e_tiles.pop(grp)
                    if VARIANT == "noexp":
                        p_tiles[grp] = dummy_p
                        return
                    if dve_exp and grp % dve_exp == 1:
                        p_sb = ppool.tile([128, 1024], I16, tag="p")
                        if EXP1OP:
                            nc.vector.tensor_scalar(
                                p_sb[:], e_prev[:], EXP_A, EXP_B,
                                mybir.AluOpType.mult, mybir.AluOpType.add,
                            )
                        else:
                            t_sb = wpool.tile([128, 1024], F32, tag="tx")
                            nc.vector.tensor_scalar(
                                t_sb[:], e_prev[:], EXP_A, EXP_B,
                                mybir.AluOpType.mult, mybir.AluOpType.add,
                            )
                            nc.vector.tensor_copy(p_sb[:], t_sb[:])
                    else:
                        p_sb = ppool.tile([128, 1024], BF, tag="p")
                        nc.scalar.activation(p_sb[:], e_prev[:], Exp)
                    p_tiles[grp] = p_sb

                # Batched software pipeline: per supergroup emit GB groups of
                # energy (PE stays in row-tiled mode), then GB exps (sg-1),
                # then GB z-runs (sg-3) — deep skew hides exp latency.
                e_tiles = {}
                p_tiles = {}
                nsg = (ng + GB - 1) // GB

                def sgroups(sgi):
                    return range(sgi * GB, min((sgi + 1) * GB, ng))

                for sg in range(nsg + 3):
                    if sg < nsg:
                        for gg in sgroups(sg):
                            e_ps = epool.tile([128, 1024], F32, tag="e")
                            e_tiles[gg] = e_ps
                            e_mms(e_ps, gg)
                    if 0 <= sg - 1 < nsg:
                        for gg in sgroups(sg - 1):
                            do_exp(gg)
                    if 0 <= sg - 3 < nsg:
                        for gg in sgroups(sg - 3):
                            pt = p_tiles.pop(gg)
                            if VARIANT != "noz":
                                z_mms(pt, gg)

                # ---- normalize + residual + store ----
                if VARIANT == "noz":
                    nc.sync.dma_start(
                        out_d[:, jb0 : jb0 + jbw], bres_sb[:, jb0 : jb0 + jbw]
                    )
                    continue
                r_sb = wpool.tile([1, jbw], F32, tag="r")
                nc.vector.reciprocal(r_sb[:], z_ps[64:65, :])
                bc_sb = wpool.tile([C, jbw], F32, tag="bc")
                nc.gpsimd.partition_broadcast(bc_sb[:], r_sb[:])
                zn_sb = wpool.tile([C, jbw], F32, tag="zn")
                nc.vector.tensor_mul(zn_sb[:], z_ps[0:64, :], bc_sb[:])
                o_sb = wpool.tile([C, jbw], F32, tag="o")
                nc.vector.tensor_add(o_sb[:], zn_sb[:], bres_sb[:, jb0 : jb0 + jbw])
                nc.sync.dma_start(out_d[:, jb0 : jb0 + jbw], o_sb[:])

    nc.compile()
    return nc


def get_program(reps=1, dve_exp=None):
    key = (reps, dve_exp)
    if key not in _PROGRAMS:
        _PROGRAMS[key] = _build_program(reps, dve_exp)
    return _PROGRAMS[key]


def make_in_maps(rv_x, bev_x, Wq, bq, Wk, bk, Wv, bv):
    rv_x = np.asarray(rv_x, np.float32)
    bev_x = np.asarray(bev_x, np.float32)
    ones = np.ones((1, N), np.float32)
    wq_ext = np.zeros((65, PADW), np.float32)
    wq_ext[:, :D] = np.concatenate([np.asarray(Wq).T, np.asarray(bq)[None]], 0)
    wq_ext = wq_ext.astype(BF16)
    wk_ext = np.zeros((65, PADW), np.float32)
    wk_ext[:, :D] = np.concatenate([np.asarray(Wk).T, np.asarray(bk)[None]], 0)
    wk_ext = wk_ext.astype(BF16)
    wv_ext = np.zeros((65, 65), np.float32)
    wv_ext[:64, :64] = np.asarray(Wv).T
    wv_ext[64, :64] = np.asarray(bv)
    wv_ext[64, 64] = 1.0
    wv_ext = wv_ext.astype(BF16)

    in_maps = []
    for core in range(NCORES):
        b = core // 4
        j0 = (core % 4) * JS
        rv2 = rv_x[b].reshape(C, N)
        bev2 = bev_x[b].reshape(C, N)
        rv_ext = np.concatenate([rv2, ones], 0).astype(BF16)
        bev_ext = np.concatenate([bev2, ones], 0).astype(BF16)
        in_maps.append(
            {
                "rv_ext": rv_ext,
                "bev_ext": bev_ext,
                "rv_slab": np.ascontiguousarray(rv_ext[:, j0 : j0 + JS]),
                "bev_res": np.ascontiguousarray(bev2[:, j0 : j0 + JS]),
                "wq_ext": wq_ext,
                "wk_ext": wk_ext,
                "wv_ext": wv_ext,
            }
        )
    return in_maps


def run(inputs, trace=False, trace_kwargs=None, reps=1, in_maps=None):
    """Run on all 8 cores; returns (output ndarray, BassKernelResults)."""
    from concourse.bass_utils import run_bass_kernel_spmd

    nc = get_program(reps)
    if in_maps is None:
        in_maps = make_in_maps(**inputs)
    res = run_bass_kernel_spmd(
        nc,
        in_maps,
        core_ids=list(range(NCORES)),
        trace=trace,
        **(trace_kwargs or {}),
    )
    out = np.zeros((B, C, N), np.float32)
    for core in range(NCORES):
        b = core // 4
        j0 = (core % 4) * JS
        out[b, :, j0 : j0 + JS] = res.results[core]["out"]
    return out.reshape(B, C, H, W), res


def kernel(**inputs):
    out, _ = run(inputs)
    return out
